# revision 35
# baseline (speedup 1.0000x reference)
"""Trainium2 Bass kernel for nn_ChimeraV2Block (dual-softmax differential
sliding-window attention block, B=1 S=2048 D=2048, 16 q-heads / 4 kv-heads,
head_dim 128, window 512).

Sharding: tensor-parallel over heads across 8 NeuronCores. Core c owns
q-heads {2c, 2c+1} and kv-head c//2 (GQA groups align with the split).
Wq/Wk/Wv column-sharded, Wo row-sharded; the 8 fp32 partial outputs are
summed on the host (the "all-reduce").

v2 structure: single software-pipelined loop. Projection sub-chunks (256
tokens) interleave with attention q-tiles so every engine keeps work:
  step c: attn_front(2c-2), attn_front(2c-1), proj(c),
          attn_back(2c-4), outproj(2c-5), attn_back(2c-3), outproj(2c-4)
attn_front = scores + dual softmax elementwise + gn transpose (DMA XBAR);
attn_back = AV matmuls + att evacuation; outproj trails one more step.
The 2-step front->back distance hides the XBAR transpose latency and the
V/S elementwise chains behind PE score/projection work, keeping the PE
p-state high. RoPE runs on bf16 SBUF copies (scalar casts, 2-byte DVE
ops at 2x rate).
"""

import sys

if "/opt/trn_rl_repo" not in sys.path:
    sys.path.insert(0, "/opt/trn_rl_repo")

import numpy as np
import ml_dtypes

BF = ml_dtypes.bfloat16

S = 2048
D = 2048
H = 16
HK = 4
HD = 128
WIN = 512
THETA = 10000.0
N_CORES = 8
NQT = S // 128          # 16 q row-tiles
NKT = D // 128          # 16 contraction tiles for the projections
WMAX = WIN + 128        # 640: max key-window width per q-tile
CH = 256                # projection sub-chunk width (tokens)
NCH = S // CH           # 8
NEG = -1.0e30

_CACHE = {}


def _tables():
    """RoPE tables [128, S] fp16 with head-dim-duplicated frequencies
    (row p uses invf[p % 64]). The sin table has the rotate-half sign
    folded in and lives at the partition of the SOURCE operand: rows
    64:128 carry -sin (read together with ps[64:128] to produce the low
    output half), rows 0:64 carry +sin. Q tables are pre-scaled by the
    attention scale 1/sqrt(64)."""
    invf = 1.0 / (THETA ** (np.arange(0, HD, 2, dtype=np.float64) / HD))  # [64]
    t = np.arange(S, dtype=np.float64)
    fr = np.outer(invf, t)  # [64, S]
    cosf = np.concatenate([np.cos(fr)] * 2, axis=0)
    sinf = np.concatenate([np.sin(fr), -np.sin(fr)], axis=0)
    return (np.ascontiguousarray(cosf * 0.125, dtype=np.float16),
            np.ascontiguousarray(sinf * 0.125, dtype=np.float16),
            np.ascontiguousarray(cosf, dtype=np.float16),
            np.ascontiguousarray(sinf, dtype=np.float16))


def _masks():
    p = np.arange(128)[:, None]
    c = np.arange(WMAX)[None, :]
    band = (c - p >= 1) & (c - p <= WIN)
    mw = np.where(band, 0.0, NEG).astype(BF)          # [128, 640]
    cc = np.arange(128)[None, :]
    mc = np.where(cc <= p, 0.0, NEG).astype(BF)       # [128, 128] causal
    # edge mask: cols [0,512) allowed, cols [512,640) causal triangle.
    # slicing the last w cols gives the mask for edge q-tiles (qi < 4).
    me = np.zeros((128, WMAX), dtype=BF)
    me[:, WIN:] = mc
    return mw, me


def _build_program():
    import concourse.bacc as bacc
    import concourse.tile as tile
    from concourse import mybir

    bf = mybir.dt.bfloat16
    f32 = mybir.dt.float32
    f16 = mybir.dt.float16
    EXP = mybir.ActivationFunctionType.Exp
    RELU = mybir.ActivationFunctionType.Relu
    MULT = mybir.AluOpType.mult
    ADD = mybir.AluOpType.add
    MAX = mybir.AluOpType.max

    nc = bacc.Bacc("TRN2", target_bir_lowering=False, debug=False,
                   num_devices=N_CORES)

    xt_d = nc.dram_tensor("xt", [128, NKT, S], bf, kind="ExternalInput")
    wq_d = nc.dram_tensor("wq", [128, NKT, 2, 128], bf, kind="ExternalInput")
    wk_d = nc.dram_tensor("wk", [128, NKT, 128], bf, kind="ExternalInput")
    wv_d = nc.dram_tensor("wv", [128, NKT, 128], bf, kind="ExternalInput")
    wo_d = nc.dram_tensor("wo", [128, 2, D], bf, kind="ExternalInput")
    lamn_d = nc.dram_tensor("lamn", [1, 2], f32, kind="ExternalInput")
    out_d = nc.dram_tensor("outp", [S, D], f16, kind="ExternalOutput")

    tqc_np, tqs_np, tkc_np, tks_np = _tables()
    mw_np, me_np = _masks()
    tqc_d = nc.inline_tensor(tqc_np, "tab_qc")
    tqs_d = nc.inline_tensor(tqs_np, "tab_qs")
    tkc_d = nc.inline_tensor(tkc_np, "tab_kc")
    tks_d = nc.inline_tensor(tks_np, "tab_ks")
    mw_d = nc.inline_tensor(mw_np, "mask_win")
    me_d = nc.inline_tensor(me_np, "mask_edge")
    idb_d = nc.inline_tensor(np.eye(128, dtype=BF), "ident_bf")

    with tile.TileContext(nc) as tc:
        with tc.tile_pool(name="xpool", bufs=1) as xp, \
             tc.tile_pool(name="wpool", bufs=1) as wp, \
             tc.tile_pool(name="pers", bufs=1) as pers, \
             tc.tile_pool(name="pj", bufs=1, space="PSUM") as pjp, \
             tc.tile_pool(name="psc", bufs=1, space="PSUM") as psc, \
             tc.tile_pool(name="pt", bufs=1) as pt, \
             tc.tile_pool(name="pse", bufs=1) as pse, \
             tc.tile_pool(name="psm", bufs=1) as psm:

            # DMA issue order matters: projection weights + first x chunk
            # first so matmuls start early; bulk x + tables stream in under
            # compute; later-phase constants (masks, wo) last.
            wq = wp.tile([128, NKT, 2, 128], bf)
            wk = wp.tile([128, NKT, 128], bf)
            wv = wp.tile([128, NKT, 128], bf)
            idb = wp.tile([128, 128], bf)
            lamn = wp.tile([1, 2], f32)

            xts = []
            for nch in range(4):
                xc = xp.tile([128, NKT, 512], bf, tag=f"xt{nch}")
                xts.append(xc)
            tqc = wp.tile([128, S], f16)
            tqs = wp.tile([128, S], f16)
            tkc = wp.tile([128, S], f16)
            tks = wp.tile([128, S], f16)

            def load_chunk(nch, ndma):
                sl = slice(nch * 512, (nch + 1) * 512)
                kstep = NKT // ndma
                for i in range(ndma):
                    ksl = slice(i * kstep, (i + 1) * kstep)
                    nc.sync.dma_start(out=xts[nch][:, ksl, :],
                                      in_=xt_d[:, ksl, sl])

            # wq + the first x half-chunk interleaved ktile-major so the
            # first projection group can start within a few microseconds
            for i in range(4):
                nc.sync.dma_start(out=wq[:, 4 * i:4 * i + 4],
                                  in_=wq_d[:, 4 * i:4 * i + 4])
                for j in range(2):
                    k2 = slice(4 * i + 2 * j, 4 * i + 2 * (j + 1))
                    nc.sync.dma_start(out=xts[0][:, k2, :],
                                      in_=xt_d[:, k2, 0:512])
            nc.sync.dma_start(out=wk[:], in_=wk_d[:])
            nc.sync.dma_start(out=wv[:], in_=wv_d[:])
            nc.sync.dma_start(out=idb[:], in_=idb_d[:])
            nc.sync.dma_start(out=lamn[:], in_=lamn_d[:])
            mw = wp.tile([128, WMAX], bf)
            nc.sync.dma_start(out=mw[:], in_=mw_d[:])
            me = wp.tile([128, WMAX], bf)
            nc.sync.dma_start(out=me[:], in_=me_d[:])
            nc.sync.dma_start(out=tqc[:], in_=tqc_d[:])
            nc.sync.dma_start(out=tqs[:], in_=tqs_d[:])
            nc.sync.dma_start(out=tkc[:], in_=tkc_d[:])
            nc.sync.dma_start(out=tks[:], in_=tks_d[:])
            load_chunk(1, 4)
            load_chunk(2, 2)
            load_chunk(3, 2)
            wo = wp.tile([128, 2, D], bf)
            nc.sync.dma_start(out=wo[:], in_=wo_d[:])
            lamb = wp.tile([128, 2], f32)
            nc.gpsimd.partition_broadcast(lamb[:], lamn[:])

            # q stored zero-padded to full 128 contraction rows per half:
            # qtp0 rows 0:64 hold half-0 q, rows 64:128 are zero; qtp1 is
            # the mirror. A 128-contraction matmul runs at 2x the column
            # rate of a 64-contraction one, so the padded zeros are free.
            qtp0 = pers.tile([128, 2, S], bf)
            qtp1 = pers.tile([128, 2, S], bf)
            kt = pers.tile([128, S], bf)         # RoPE'd k, hd-major
            vsm = pers.tile([128, NQT, 128], bf)  # v, S-major [s, hd]
            att = pers.tile([128, 2, S], bf)     # attention out^T, hd-major
            nc.gpsimd.memset(qtp0[64:128, :, :], 0.0)
            nc.gpsimd.memset(qtp1[0:64, :, :], 0.0)

            state = {}

            def _rope(ps, idx, sl, outlo, outhi, tabc, tabs, tg):
                # cast PSUM -> SBUF bf16 on the scalar engine, then all
                # RoPE math is 2-byte DVE work at 2x rate.
                psb = pt.tile([128, CH], bf, tag="psb" + tg, bufs=2,
                              name="psb")
                nc.scalar.copy(out=psb[:], in_=ps[:, idx, :])
                m1 = pt.tile([128, CH], bf, tag="m1" + tg, bufs=2, name="m1")
                m2 = pt.tile([128, CH], bf, tag="m2" + tg, bufs=2, name="m2")
                nc.vector.tensor_mul(m1[:], psb[:], tabc[:, sl])
                nc.vector.tensor_mul(m2[0:64, :], psb[64:128, :], tabs[64:128, sl])
                nc.vector.tensor_mul(m2[64:128, :], psb[0:64, :], tabs[0:64, sl])
                if outlo is None:
                    nc.vector.tensor_add(kt[:, sl], m1[:], m2[:])
                else:
                    nc.vector.tensor_add(outlo, m1[0:64, :], m2[0:64, :])
                    nc.vector.tensor_add(outhi, m1[64:128, :], m2[64:128, :])

            def _pjmm(ps, idx, xi, xoff):
                # one accumulation group at a time per PSUM region: groups
                # sharing a bank must not be concurrently active
                for kti in range(NKT):
                    st = kti == 0
                    sp = kti == NKT - 1
                    rhs = xts[xi][:, kti, xoff:xoff + CH]
                    if idx == 0:
                        lhsT = wq[:, kti, 0, :]
                    elif idx == 1:
                        lhsT = wq[:, kti, 1, :]
                    elif idx == 2:
                        lhsT = wk[:, kti, :]
                    else:
                        lhsT = wv[:, kti, :]
                    nc.tensor.matmul(ps[:, idx, :], lhsT, rhs, start=st, stop=sp)

            def proj_a(c):
                """Project tokens [256c, 256c+256): q heads + their RoPE."""
                sl = slice(c * CH, (c + 1) * CH)
                xi, xoff = c // 2, (c % 2) * CH
                ps = pjp.tile([128, 4, CH], f32, tag="pj", bufs=1)
                state[("pj", c)] = ps
                _pjmm(ps, 0, xi, xoff)
                _pjmm(ps, 1, xi, xoff)
                _rope(ps, 0, sl, qtp0[0:64, 0, sl], qtp1[64:128, 0, sl],
                      tqc, tqs, "q0")
                _rope(ps, 1, sl, qtp0[0:64, 1, sl], qtp1[64:128, 1, sl],
                      tqc, tqs, "q1")

            def proj_b(c):
                """k/v projections for the chunk + k RoPE + v transpose."""
                sl = slice(c * CH, (c + 1) * CH)
                xi, xoff = c // 2, (c % 2) * CH
                ps = state[("pj", c)]
                _pjmm(ps, 2, xi, xoff)
                _pjmm(ps, 3, xi, xoff)
                _rope(ps, 2, sl, None, None, tkc, tks, "k")
                vtmp = pt.tile([128, CH], bf, tag="vtmp", bufs=2)
                nc.vector.tensor_copy(out=vtmp[:], in_=ps[:, 3, :])
                nc.sync.dma_start(out=vsm[:, 2 * c:2 * c + 2, :], in_=vtmp[:],
                                  transpose=True)

            def attn_front(qi):
                """Scores + dual-softmax elementwise + gn transpose issue."""
                qsl = slice(qi * 128, (qi + 1) * 128)
                kw = min(qi + 1, 5)
                w = kw * 128
                kstart = max(0, qi - 4)
                kwin = slice(kstart * 128, kstart * 128 + w)

                gts = pse.tile([128, 2, 5, 128], bf, tag="gts", bufs=5)
                state[qi] = (gts, kw, kstart)

                for h in range(2):
                    ps_s1 = psc.tile([128, WMAX], f32, tag="s", bufs=2)
                    ps_s2 = psc.tile([128, WMAX], f32, tag="s", bufs=2)
                    for ps, lhsq in ((ps_s1, qtp0), (ps_s2, qtp1)):
                        lhs = lhsq[:, h, qsl]
                        if qi >= 4:
                            # score matmuls first (512-col ISA limit per
                            # instruction), then band-edge mask adds on
                            # blocks 0 and 4 only
                            nc.tensor.matmul(ps[:, 0:512], lhs,
                                             kt[:, kwin][:, 0:512],
                                             start=True, stop=False,
                                             skip_group_check=True)
                            nc.tensor.matmul(ps[:, 512:640], idb[:],
                                             mw[:, 512:640],
                                             start=True, stop=False,
                                             skip_group_check=True)
                            nc.tensor.matmul(ps[:, 512:640], lhs,
                                             kt[:, kwin][:, 512:640],
                                             start=False, stop=False,
                                             skip_group_check=True)
                            nc.tensor.matmul(ps[:, 0:128], idb[:], mw[:, 0:128],
                                             start=False, stop=True,
                                             skip_group_check=True)
                        else:
                            nc.tensor.matmul(ps[:, 0:w], idb[:],
                                             me[:, WMAX - w:WMAX],
                                             start=True, stop=False)
                            nc.tensor.matmul(ps[:, 0:w], lhs,
                                             kt[:, kwin][:, 0:w],
                                             start=False, stop=True)

                    e1 = pse.tile([128, WMAX], bf, tag="e1", bufs=4)
                    e2 = pse.tile([128, WMAX], bf, tag="e2", bufs=4)
                    s1 = psm.tile([128, 1], f32, tag="s1", bufs=4)
                    s2 = psm.tile([128, 1], f32, tag="s2", bufs=4)
                    nc.scalar.activation(out=e1[:, 0:w], in_=ps_s1[:, 0:w],
                                         func=EXP, accum_out=s1[:])
                    nc.scalar.activation(out=e2[:, 0:w], in_=ps_s2[:, 0:w],
                                         func=EXP, accum_out=s2[:])

                    # cneg = -(lam * s1 / s2)   (lamn holds -lam)
                    r2 = psm.tile([128, 1], f32, tag="r2", bufs=4)
                    nc.vector.reciprocal(out=r2[:], in_=s2[:])
                    cneg = psm.tile([128, 1], f32, tag="cneg", bufs=4)
                    nc.vector.scalar_tensor_tensor(
                        out=cneg[:], in0=s1[:], scalar=lamb[:, h:h + 1],
                        in1=r2[:], op0=MULT, op1=MULT)
                    # g0 = e1 + cneg*e2 ; g = relu(g0), accum D'
                    g0 = pse.tile([128, WMAX], bf, tag="g0", bufs=3)
                    g = pse.tile([128, WMAX], bf, tag="g", bufs=4)
                    dsum = psm.tile([128, 1], f32, tag="dsum", bufs=4)
                    nc.vector.scalar_tensor_tensor(
                        out=g0[:, 0:w], in0=e2[:, 0:w], scalar=cneg[:],
                        in1=e1[:, 0:w], op0=MULT, op1=ADD)
                    if h == 0:
                        nc.scalar.activation(
                            out=g[:, 0:w], in_=g0[:, 0:w], func=RELU,
                            accum_out=dsum[:])
                    else:
                        nc.vector.tensor_scalar(
                            out=g[:, 0:w], in0=g0[:, 0:w], scalar1=0.0,
                            scalar2=0.0, op0=MAX, op1=ADD, accum_out=dsum[:])
                    # recd = 1 / (D' + 1e-6 * s1); gn = g * recd
                    dtmp = psm.tile([128, 1], f32, tag="dtmp", bufs=4)
                    nc.vector.scalar_tensor_tensor(
                        out=dtmp[:], in0=s1[:], scalar=1e-6, in1=dsum[:],
                        op0=MULT, op1=ADD)
                    recd = psm.tile([128, 1], f32, tag="recd", bufs=4)
                    nc.vector.reciprocal(out=recd[:], in_=dtmp[:])
                    if h == 0:
                        gn2 = pse.tile([128, 2, WMAX], bf, tag="gn", bufs=4)
                    nc.vector.tensor_scalar(
                        out=gn2[:, h, 0:w], in0=g[:, 0:w], scalar1=recd[:],
                        scalar2=0.0, op0=MULT, op1=ADD)
                # XBAR transpose; the consumer (attn_back) runs ~1.5
                # pipeline steps later, hiding the transfer latency. Full
                # tiles combine both heads into one transfer (the input
                # slice is contiguous only when w == WMAX).
                if w == WMAX:
                    nc.sync.dma_start(out=gts[:, :, 0:kw, :],
                                      in_=gn2[:, :, 0:w], transpose=True)
                else:
                    nc.sync.dma_start(out=gts[:, 0, 0:kw, :],
                                      in_=gn2[:, 0, 0:w], transpose=True)
                    nc.sync.dma_start(out=gts[:, 1, 0:kw, :],
                                      in_=gn2[:, 1, 0:w], transpose=True)

            def attn_back(qi):
                """AV for both heads at once: [k,hd]^T-contract x [k, 2*128]."""
                gts, kw, kstart = state[qi]
                qsl = slice(qi * 128, (qi + 1) * 128)
                ps_av = psc.tile([128, 2, 128], f32, tag="av", bufs=1)
                for j in range(kw):
                    nc.tensor.matmul(ps_av[:], vsm[:, kstart + j, :],
                                     gts[:, :, j, :],
                                     start=(j == 0), stop=(j == kw - 1))
                nc.vector.tensor_copy(out=att[:, :, qsl], in_=ps_av[:])

            def outproj(qi, dch):
                """One 512-col out-projection slice; slices are spread
                across schedule slots so the PSUM evacuation copy of
                slice n drains behind other PE work before slice n+1
                reuses the bank."""
                qsl = slice(qi * 128, (qi + 1) * 128)
                if dch == 0:
                    so = pse.tile([128, 2048], f16, tag="so", bufs=3,
                                  name="so")
                    state[("so", qi)] = so
                so = state[("so", qi)]
                dsl = slice(dch * 512, (dch + 1) * 512)
                ps_o = psc.tile([128, 512], f32, tag="o", bufs=1)
                nc.tensor.matmul(ps_o[:], att[:, 0, qsl], wo[:, 0, dsl],
                                 start=True, stop=False)
                nc.tensor.matmul(ps_o[:], att[:, 1, qsl], wo[:, 1, dsl],
                                 start=False, stop=True)
                if dch % 2 == 0:
                    nc.vector.tensor_copy(out=so[:, dsl], in_=ps_o[:])
                else:
                    nc.scalar.copy(out=so[:, dsl], in_=ps_o[:])
                if dch == 3:
                    nc.sync.dma_start(out=out_d[qsl, :], in_=so[:])

            # ---- software-pipelined schedule ----
            # fronts run 2 proj-steps behind the projection that feeds
            # them (slack for RoPE + weight-load ordering); backs 1 step
            # behind fronts; out-projection slices trail one more step and
            # are spread through the step so their PSUM copies drain
            # behind score/projection matmuls.
            acts = []
            for c in range(NCH):
                j0, j1 = 2 * c - 8, 2 * c - 7
                acts += [("f", 2 * c - 4), ("o", j0, 0), ("pa", c),
                         ("o", j1, 0), ("f", 2 * c - 3), ("o", j0, 1),
                         ("pb", c), ("o", j1, 1),
                         ("b", 2 * c - 6), ("o", j0, 2),
                         ("b", 2 * c - 5), ("o", j1, 2),
                         ("o", j0, 3), ("o", j1, 3)]
            acts += [("f", 12, 0), ("o", 8, 0), ("o", 8, 1),
                     ("f", 13, 0), ("o", 8, 2), ("o", 8, 3),
                     ("b", 10, 0), ("o", 9, 0), ("o", 9, 1),
                     ("b", 11, 0), ("o", 9, 2), ("o", 9, 3),
                     ("f", 14, 0), ("o", 10, 0), ("o", 10, 1),
                     ("f", 15, 0), ("o", 10, 2), ("o", 10, 3),
                     ("b", 12, 0), ("o", 11, 0), ("o", 11, 1),
                     ("b", 13, 0), ("o", 11, 2), ("o", 11, 3),
                     ("b", 14, 0), ("o", 12, 0), ("o", 12, 1),
                     ("b", 15, 0), ("o", 12, 2), ("o", 12, 3)]
            for j in range(13, 16):
                acts += [("o", j, 0), ("o", j, 1), ("o", j, 2), ("o", j, 3)]
            for act in acts:
                kind, j = act[0], act[1]
                if j is None or j < 0:
                    continue
                if kind == "f":
                    attn_front(j)
                elif kind == "pa":
                    proj_a(j)
                elif kind == "pb":
                    proj_b(j)
                elif kind == "b":
                    attn_back(j)
                else:
                    outproj(j, act[2])

    nc.compile()
    return nc


def get_program():
    if "nc" not in _CACHE:
        _CACHE["nc"] = _build_program()
    return _CACHE["nc"]


def _prep_inputs(x, Wq, Wk, Wv, Wo, lam):
    xt = np.ascontiguousarray(x.reshape(S, D).T.astype(BF)
                              .reshape(NKT, 128, S).transpose(1, 0, 2))
    in_maps = []
    for c in range(N_CORES):
        h0 = 2 * c
        kv = c // 2
        wq_c = np.ascontiguousarray(
            Wq[:, h0 * 128:(h0 + 2) * 128].astype(BF)
            .reshape(NKT, 128, 2, 128).transpose(1, 0, 2, 3))
        wk_c = np.ascontiguousarray(
            Wk[:, kv * 128:(kv + 1) * 128].astype(BF)
            .reshape(NKT, 128, 128).transpose(1, 0, 2))
        wv_c = np.ascontiguousarray(
            Wv[:, kv * 128:(kv + 1) * 128].astype(BF)
            .reshape(NKT, 128, 128).transpose(1, 0, 2))
        wo_c = np.ascontiguousarray(
            Wo[h0 * 128:(h0 + 2) * 128, :].astype(BF)
            .reshape(2, 128, D).transpose(1, 0, 2))
        lamn_c = np.array([[-float(lam[h0]), -float(lam[h0 + 1])]], dtype=np.float32)
        in_maps.append({"xt": xt, "wq": wq_c, "wk": wk_c, "wv": wv_c,
                        "wo": wo_c, "lamn": lamn_c})
    return in_maps


def kernel(x, Wq, Wk, Wv, Wo, lam):
    from concourse.bass_utils import run_bass_kernel_spmd

    nc = get_program()
    in_maps = _prep_inputs(np.asarray(x), np.asarray(Wq), np.asarray(Wk),
                           np.asarray(Wv), np.asarray(Wo), np.asarray(lam))
    res = run_bass_kernel_spmd(nc, in_maps, list(range(N_CORES)))
    out = np.zeros((S, D), dtype=np.float32)
    for c in range(N_CORES):
        out += res.results[c]["outp"].astype(np.float32)
    return out.reshape(1, S, D)


# revision 37
# speedup vs baseline: 1.0436x; 1.0436x over previous
"""Trainium2 Bass kernel for nn_ChimeraV2Block (dual-softmax differential
sliding-window attention block, B=1 S=2048 D=2048, 16 q-heads / 4 kv-heads,
head_dim 128, window 512).

Sharding: tensor-parallel over heads across 8 NeuronCores. Core c owns
q-heads {2c, 2c+1} and kv-head c//2 (GQA groups align with the split).
Wq/Wk/Wv column-sharded, Wo row-sharded; the 8 fp32 partial outputs are
summed on the host (the "all-reduce").
"""

import sys

if "/opt/trn_rl_repo" not in sys.path:
    sys.path.insert(0, "/opt/trn_rl_repo")

import numpy as np
import ml_dtypes

BF = ml_dtypes.bfloat16

S = 2048
D = 2048
H = 16
HK = 4
HD = 128
WIN = 512
THETA = 10000.0
N_CORES = 8
NQT = S // 128          # 16 q row-tiles
NKT = D // 128          # 16 contraction tiles for the projections
WMAX = WIN + 128        # 640: max key-window width per q-tile
NEG = -1.0e30

_CACHE = {}


def _tables():
    """RoPE tables [128, S] fp16 with head-dim-duplicated frequencies
    (row p uses invf[p % 64]). The sin table has the rotate-half sign
    folded in and lives at the partition of the SOURCE operand: rows
    64:128 carry -sin (read together with ps[64:128] to produce the low
    output half), rows 0:64 carry +sin. Q tables are pre-scaled by the
    attention scale 1/sqrt(64)."""
    invf = 1.0 / (THETA ** (np.arange(0, HD, 2, dtype=np.float64) / HD))  # [64]
    t = np.arange(S, dtype=np.float64)
    fr = np.outer(invf, t)  # [64, S]
    cosf = np.concatenate([np.cos(fr)] * 2, axis=0)
    sinf = np.concatenate([np.sin(fr), -np.sin(fr)], axis=0)
    return (np.ascontiguousarray(cosf * 0.125, dtype=np.float16),
            np.ascontiguousarray(sinf * 0.125, dtype=np.float16),
            np.ascontiguousarray(cosf, dtype=np.float16),
            np.ascontiguousarray(sinf, dtype=np.float16))


def _masks():
    p = np.arange(128)[:, None]
    c = np.arange(WMAX)[None, :]
    band = (c - p >= 1) & (c - p <= WIN)
    mw = np.where(band, 0.0, NEG).astype(BF)          # [128, 640]
    cc = np.arange(128)[None, :]
    mc = np.where(cc <= p, 0.0, NEG).astype(BF)       # [128, 128] causal
    # edge mask: cols [0,512) allowed, cols [512,640) causal triangle.
    # slicing the last w cols gives the mask for edge q-tiles (qi < 4).
    me = np.zeros((128, WMAX), dtype=BF)
    me[:, WIN:] = mc
    return mw, me


def _build_program():
    import concourse.bacc as bacc
    import concourse.tile as tile
    from concourse import mybir

    bf = mybir.dt.bfloat16
    f32 = mybir.dt.float32
    EXP = mybir.ActivationFunctionType.Exp
    RELU = mybir.ActivationFunctionType.Relu
    MULT = mybir.AluOpType.mult
    ADD = mybir.AluOpType.add
    MAX = mybir.AluOpType.max
    DIV = mybir.AluOpType.divide

    nc = bacc.Bacc("TRN2", target_bir_lowering=False, debug=False,
                   num_devices=N_CORES)

    xt_d = nc.dram_tensor("xt", [128, NKT, S], bf, kind="ExternalInput")
    wq_d = nc.dram_tensor("wq", [128, NKT, 2, 128], bf, kind="ExternalInput")
    wk_d = nc.dram_tensor("wk", [128, NKT, 128], bf, kind="ExternalInput")
    wv_d = nc.dram_tensor("wv", [128, NKT, 128], bf, kind="ExternalInput")
    wo_d = nc.dram_tensor("wo", [128, 2, D], bf, kind="ExternalInput")
    lamn_d = nc.dram_tensor("lamn", [1, 2], f32, kind="ExternalInput")
    f16 = mybir.dt.float16
    out_d = nc.dram_tensor("outp", [S, D], f16, kind="ExternalOutput")

    tqc_np, tqs_np, tkc_np, tks_np = _tables()
    mw_np, me_np = _masks()
    tqc_d = nc.inline_tensor(tqc_np, "tab_qc")
    tqs_d = nc.inline_tensor(tqs_np, "tab_qs")
    tkc_d = nc.inline_tensor(tkc_np, "tab_kc")
    tks_d = nc.inline_tensor(tks_np, "tab_ks")
    mw_d = nc.inline_tensor(mw_np, "mask_win")
    me_d = nc.inline_tensor(me_np, "mask_edge")
    idb_d = nc.inline_tensor(np.eye(128, dtype=BF), "ident_bf")
    idf_d = nc.inline_tensor(np.eye(128, dtype=np.float32), "ident_f32")

    with tile.TileContext(nc) as tc:
        with tc.tile_pool(name="xpool", bufs=1) as xp, \
             tc.tile_pool(name="wpool", bufs=1) as wp, \
             tc.tile_pool(name="pers", bufs=1) as pers:

            # DMA issue order matters: projection weights + first x chunk
            # first so matmuls start early; bulk x + tables stream in under
            # compute; phase-2/3 constants (masks, wo) last.
            wq = wp.tile([128, NKT, 2, 128], bf)
            wk = wp.tile([128, NKT, 128], bf)
            nc.sync.dma_start(out=wk[:, 0:8], in_=wk_d[:, 0:8])
            nc.sync.dma_start(out=wk[:, 8:16], in_=wk_d[:, 8:16])
            wv = wp.tile([128, NKT, 128], bf)
            nc.sync.dma_start(out=wv[:, 0:8], in_=wv_d[:, 0:8])
            nc.sync.dma_start(out=wv[:, 8:16], in_=wv_d[:, 8:16])
            idb = wp.tile([128, 128], bf)
            nc.sync.dma_start(out=idb[:], in_=idb_d[:])
            lamn = wp.tile([1, 2], f32)
            nc.sync.dma_start(out=lamn[:], in_=lamn_d[:])

            xts = []
            for nch in range(4):
                xc = xp.tile([128, NKT, 512], bf, tag=f"xt{nch}")
                xts.append(xc)
            tqc = wp.tile([128, S], f16)
            tqs = wp.tile([128, S], f16)
            tkc = wp.tile([128, S], f16)
            tks = wp.tile([128, S], f16)

            def load_chunk(nch, ndma):
                sl = slice(nch * 512, (nch + 1) * 512)
                kstep = NKT // ndma
                for i in range(ndma):
                    ksl = slice(i * kstep, (i + 1) * kstep)
                    nc.sync.dma_start(out=xts[nch][:, ksl, :],
                                      in_=xt_d[:, ksl, sl])

            for i in range(4):
                nc.sync.dma_start(out=wq[:, 4 * i:4 * i + 4],
                                  in_=wq_d[:, 4 * i:4 * i + 4])
                ksl = slice(4 * i, 4 * (i + 1))
                for j in range(2):
                    k2 = slice(4 * i + 2 * j, 4 * i + 2 * (j + 1))
                    nc.sync.dma_start(out=xts[0][:, k2, :],
                                      in_=xt_d[:, k2, 0:512])
            for i in range(2):
                sl = slice(1024 * i, 1024 * (i + 1))
                nc.sync.dma_start(out=tqc[:, sl], in_=tqc_d[:, sl])
                nc.sync.dma_start(out=tqs[:, sl], in_=tqs_d[:, sl])
                nc.sync.dma_start(out=tkc[:, sl], in_=tkc_d[:, sl])
                nc.sync.dma_start(out=tks[:, sl], in_=tks_d[:, sl])
            load_chunk(1, 8)
            load_chunk(2, 4)
            load_chunk(3, 4)
            mw = wp.tile([128, WMAX], bf)
            nc.sync.dma_start(out=mw[:], in_=mw_d[:])
            me = wp.tile([128, WMAX], bf)
            nc.sync.dma_start(out=me[:], in_=me_d[:])
            wo = wp.tile([128, 2, D], bf)
            for i in range(4):
                nc.sync.dma_start(out=wo[:, :, 512 * i:512 * (i + 1)],
                                  in_=wo_d[:, :, 512 * i:512 * (i + 1)])
            lamb = wp.tile([128, 2], f32)
            nc.gpsimd.partition_broadcast(lamb[:], lamn[:])

            # q stored zero-padded to full 128 contraction rows per half:
            # qtp0 rows 0:64 hold half-0 q, rows 64:128 are zero; qtp1 is
            # the mirror. A 128-contraction matmul runs at 2x the column
            # rate of a 64-contraction one, so the padded zeros are free.
            qtp0 = pers.tile([128, 2, S], bf)
            qtp1 = pers.tile([128, 2, S], bf)
            kt = pers.tile([128, S], bf)         # RoPE'd k, hd-major
            vsm = pers.tile([128, NQT, 128], bf)  # v, S-major [s, hd]
            att = pers.tile([128, 2, S], bf)     # attention out^T, hd-major
            zeros = pers.tile([128, WMAX], bf)
            nc.gpsimd.memset(qtp0[64:128, :, :], 0.0)
            nc.gpsimd.memset(qtp1[0:64, :, :], 0.0)
            nc.gpsimd.memset(zeros[:], 0.0)

            # ---- Phase 1: projections + RoPE + v transpose ----
            with tc.tile_pool(name="pp", bufs=1, space="PSUM") as pp, \
                 tc.tile_pool(name="pt", bufs=2) as pt:
                for nch in range(4):
                    sl = slice(nch * 512, (nch + 1) * 512)
                    ps_q0 = pp.tile([128, 512], f32, tag="pq0", bufs=2)
                    ps_q1 = pp.tile([128, 512], f32, tag="pq1", bufs=2)
                    ps_k = pp.tile([128, 512], f32, tag="pk", bufs=1)
                    ps_v = pp.tile([128, 512], f32, tag="pv", bufs=1)
                    for kti in range(NKT):
                        st = kti == 0
                        sp = kti == NKT - 1
                        rhs = xts[nch][:, kti, :]
                        nc.tensor.matmul(ps_q0[:], wq[:, kti, 0, :], rhs, start=st, stop=sp)
                        nc.tensor.matmul(ps_q1[:], wq[:, kti, 1, :], rhs, start=st, stop=sp)
                        nc.tensor.matmul(ps_k[:], wk[:, kti, :], rhs, start=st, stop=sp)
                        nc.tensor.matmul(ps_v[:], wv[:, kti, :], rhs, start=st, stop=sp)
                    for ps, outlo, outhi, tabc, tabs, tg in (
                            (ps_q0, qtp0[0:64, 0, sl], qtp1[64:128, 0, sl], tqc, tqs, "q0"),
                            (ps_q1, qtp0[0:64, 1, sl], qtp1[64:128, 1, sl], tqc, tqs, "q1"),
                            (ps_k, None, None, tkc, tks, "k")):
                        # out = ps*cos + rot_half(ps)*sin, sign folded in
                        # tabs. The PSUM is cast to bf16 SBUF on the (idle
                        # in phase 1) scalar engine so the DVE muls/adds
                        # run in 2-byte 2x mode.
                        psb = pt.tile([128, 512], bf, tag="psb" + tg,
                                      name="psb")
                        nc.scalar.copy(out=psb[:], in_=ps[:])
                        m1 = pt.tile([128, 512], bf, tag="m1" + tg, name="m1")
                        m2 = pt.tile([128, 512], bf, tag="m2" + tg, name="m2")
                        nc.vector.tensor_mul(m1[:], psb[:], tabc[:, sl])
                        nc.vector.tensor_mul(m2[0:64, :], psb[64:128, :], tabs[64:128, sl])
                        nc.vector.tensor_mul(m2[64:128, :], psb[0:64, :], tabs[0:64, sl])
                        if outlo is None:
                            nc.vector.tensor_add(kt[:, sl], m1[:], m2[:])
                        else:
                            nc.vector.tensor_add(outlo, m1[0:64, :], m2[0:64, :])
                            nc.vector.tensor_add(outhi, m1[64:128, :], m2[64:128, :])
                    vtmp = pt.tile([128, 512], bf, tag="vtmp")
                    nc.vector.tensor_copy(out=vtmp[:], in_=ps_v[:])
                    nc.sync.dma_start(out=vsm[:, 4 * nch:4 * (nch + 1), :],
                                      in_=vtmp[:], transpose=True)

            # ---- Phase 2: attention ----
            with tc.tile_pool(name="psc", bufs=1, space="PSUM") as psc, \
                 tc.tile_pool(name="pse", bufs=1) as pse, \
                 tc.tile_pool(name="psm", bufs=1) as psm:
                for qi in range(NQT):
                    qsl = slice(qi * 128, (qi + 1) * 128)
                    kw = min(qi + 1, 5)
                    w = kw * 128
                    kstart = max(0, qi - 4)
                    kwin = slice(kstart * 128, kstart * 128 + w)

                    # both heads' AV share one 256-wide PSUM tile
                    ps_av = psc.tile([128, 2, 128], f32, tag="av", bufs=1)
                    gts = pse.tile([128, 2, 5, 128], bf, tag="gts", bufs=2)

                    for h in range(2):
                        ps_s1 = psc.tile([128, WMAX], f32, tag="s", bufs=2)
                        ps_s2 = psc.tile([128, WMAX], f32, tag="s", bufs=2)
                        for ps, lhsq in ((ps_s1, qtp0), (ps_s2, qtp1)):
                            lhs = lhsq[:, h, qsl]
                            if qi >= 4:
                                # causal edge lives only in block 0, so the
                                # mask matmul covers 128 cols; the score
                                # matmul splits at that boundary
                                nc.tensor.matmul(ps[:, 0:128], idb[:], mw[:, 0:128],
                                                 start=True, stop=False)
                                nc.tensor.matmul(ps[:, 0:128], lhs,
                                                 kt[:, kwin][:, 0:128],
                                                 start=False, stop=True)
                                nc.tensor.matmul(ps[:, 128:512], lhs,
                                                 kt[:, kwin][:, 128:512],
                                                 start=True, stop=True)
                                nc.tensor.matmul(ps[:, 512:640], idb[:],
                                                 mw[:, 512:640],
                                                 start=True, stop=False)
                                nc.tensor.matmul(ps[:, 512:640], lhs,
                                                 kt[:, kwin][:, 512:640],
                                                 start=False, stop=True)
                            else:
                                nc.tensor.matmul(ps[:, 0:w], idb[:],
                                                 me[:, WMAX - w:WMAX],
                                                 start=True, stop=False)
                                nc.tensor.matmul(ps[:, 0:w], lhs,
                                                 kt[:, kwin][:, 0:w],
                                                 start=False, stop=True)

                        e1 = pse.tile([128, WMAX], bf, tag="e1", bufs=2)
                        e2 = pse.tile([128, WMAX], bf, tag="e2", bufs=2)
                        s1 = psm.tile([128, 1], f32, tag="s1", bufs=4)
                        s2 = psm.tile([128, 1], f32, tag="s2", bufs=4)
                        nc.scalar.activation(out=e1[:, 0:w], in_=ps_s1[:, 0:w],
                                             func=EXP, accum_out=s1[:])
                        nc.scalar.activation(out=e2[:, 0:w], in_=ps_s2[:, 0:w],
                                             func=EXP, accum_out=s2[:])

                        # cneg = -(lam * s1 / s2)   (lamn holds -lam)
                        r2 = psm.tile([128, 1], f32, tag="r2", bufs=4)
                        nc.vector.reciprocal(out=r2[:], in_=s2[:])
                        cneg = psm.tile([128, 1], f32, tag="cneg", bufs=4)
                        nc.vector.scalar_tensor_tensor(
                            out=cneg[:], in0=s1[:], scalar=lamb[:, h:h + 1],
                            in1=r2[:], op0=MULT, op1=MULT)
                        # e2c = cneg*e2 (DVE 2-byte 2x rate); then
                        # g0 = e1 + e2c on the otherwise-idle GpSimd;
                        # g = relu(g0), accum D'
                        e2c = pse.tile([128, WMAX], bf, tag="e2c", bufs=2)
                        nc.vector.tensor_scalar(
                            out=e2c[:, 0:w], in0=e2[:, 0:w], scalar1=cneg[:],
                            scalar2=0.0, op0=MULT, op1=ADD)
                        g0 = pse.tile([128, WMAX], bf, tag="g0", bufs=2)
                        g = pse.tile([128, WMAX], bf, tag="g", bufs=2)
                        dsum = psm.tile([128, 1], f32, tag="dsum", bufs=4)
                        nc.gpsimd.tensor_tensor(
                            out=g0[:, 0:w], in0=e1[:, 0:w], in1=e2c[:, 0:w],
                            op=ADD)
                        if h == 0:
                            nc.scalar.activation(
                                out=g[:, 0:w], in_=g0[:, 0:w], func=RELU,
                                accum_out=dsum[:])
                        else:
                            nc.vector.tensor_scalar(
                                out=g[:, 0:w], in0=g0[:, 0:w], scalar1=0.0,
                                scalar2=0.0, op0=MAX, op1=ADD, accum_out=dsum[:])
                        # recd = 1 / (D' + 1e-6 * s1); gn = g * recd
                        dtmp = psm.tile([128, 1], f32, tag="dtmp", bufs=4)
                        nc.vector.scalar_tensor_tensor(
                            out=dtmp[:], in0=s1[:], scalar=1e-6, in1=dsum[:],
                            op0=MULT, op1=ADD)
                        recd = psm.tile([128, 1], f32, tag="recd", bufs=4)
                        nc.vector.reciprocal(out=recd[:], in_=dtmp[:])
                        gn = pse.tile([128, WMAX], bf, tag="gn", bufs=2)
                        nc.vector.tensor_scalar(
                            out=gn[:, 0:w], in0=g[:, 0:w], scalar1=recd[:],
                            scalar2=0.0, op0=MULT, op1=ADD)

                        # transpose gn -> gT (PSUM) -> SBUF
                        ps_tr = psc.tile([128, kw, 128], bf, tag="trg", bufs=2)
                        for j in range(kw):
                            nc.tensor.transpose(ps_tr[:, j, :],
                                                gn[:, 128 * j:128 * (j + 1)], idb[:])
                        if h == 0:
                            nc.vector.tensor_copy(out=gts[:, 0, 0:kw, :], in_=ps_tr[:])
                        else:
                            nc.scalar.copy(out=gts[:, 1, 0:kw, :], in_=ps_tr[:])

                    # AV for both heads at once: [k,hd]^T-contract x [k, 2*128]
                    for j in range(kw):
                        nc.tensor.matmul(ps_av[:], vsm[:, kstart + j, :],
                                         gts[:, :, j, :],
                                         start=(j == 0), stop=(j == kw - 1))

                    nc.vector.tensor_copy(out=att[:, :, qsl], in_=ps_av[:])

                    # out-projection for this q-tile, interleaved so the PE
                    # fills attention bubbles and the output DMA spreads out
                    so = pse.tile([128, 2048], f16, tag="so", bufs=2)
                    for dch in range(4):
                        dsl = slice(dch * 512, (dch + 1) * 512)
                        ps_o = psc.tile([128, 512], f32, tag="o", bufs=1)
                        nc.tensor.matmul(ps_o[:], att[:, 0, qsl], wo[:, 0, dsl],
                                         start=True, stop=False)
                        nc.tensor.matmul(ps_o[:], att[:, 1, qsl], wo[:, 1, dsl],
                                         start=False, stop=True)
                        if dch % 2 == 0:
                            nc.vector.tensor_copy(out=so[:, dsl], in_=ps_o[:])
                        else:
                            nc.scalar.copy(out=so[:, dsl], in_=ps_o[:])
                        if dch % 2 == 1:
                            dsl2 = slice((dch - 1) * 512, (dch + 1) * 512)
                            nc.sync.dma_start(out=out_d[qsl, dsl2], in_=so[:, dsl2])

    nc.compile()
    return nc


def get_program():
    if "nc" not in _CACHE:
        _CACHE["nc"] = _build_program()
    return _CACHE["nc"]


def _prep_inputs(x, Wq, Wk, Wv, Wo, lam):
    xt = np.ascontiguousarray(x.reshape(S, D).T.astype(BF)
                              .reshape(NKT, 128, S).transpose(1, 0, 2))
    in_maps = []
    for c in range(N_CORES):
        h0 = 2 * c
        kv = c // 2
        wq_c = np.ascontiguousarray(
            Wq[:, h0 * 128:(h0 + 2) * 128].astype(BF)
            .reshape(NKT, 128, 2, 128).transpose(1, 0, 2, 3))
        wk_c = np.ascontiguousarray(
            Wk[:, kv * 128:(kv + 1) * 128].astype(BF)
            .reshape(NKT, 128, 128).transpose(1, 0, 2))
        wv_c = np.ascontiguousarray(
            Wv[:, kv * 128:(kv + 1) * 128].astype(BF)
            .reshape(NKT, 128, 128).transpose(1, 0, 2))
        wo_c = np.ascontiguousarray(
            Wo[h0 * 128:(h0 + 2) * 128, :].astype(BF)
            .reshape(2, 128, D).transpose(1, 0, 2))
        lamn_c = np.array([[-float(lam[h0]), -float(lam[h0 + 1])]], dtype=np.float32)
        in_maps.append({"xt": xt, "wq": wq_c, "wk": wk_c, "wv": wv_c,
                        "wo": wo_c, "lamn": lamn_c})
    return in_maps


def kernel(x, Wq, Wk, Wv, Wo, lam):
    from concourse.bass_utils import run_bass_kernel_spmd

    nc = get_program()
    in_maps = _prep_inputs(np.asarray(x), np.asarray(Wq), np.asarray(Wk),
                           np.asarray(Wv), np.asarray(Wo), np.asarray(lam))
    res = run_bass_kernel_spmd(nc, in_maps, list(range(N_CORES)))
    out = np.zeros((S, D), dtype=np.float32)
    for c in range(N_CORES):
        out += res.results[c]["outp"].astype(np.float32)
    return out.reshape(1, S, D)



# revision 38
# speedup vs baseline: 1.1806x; 1.1313x over previous
"""Trainium2 Bass kernel for nn_ChimeraV2Block (dual-softmax differential
sliding-window attention block, B=1 S=2048 D=2048, 16 q-heads / 4 kv-heads,
head_dim 128, window 512).

Sharding: tensor-parallel over heads across 8 NeuronCores. Core c owns
q-heads {2c, 2c+1} and kv-head c//2 (GQA groups align with the split).
Wq/Wk/Wv column-sharded, Wo row-sharded; the 8 fp32 partial outputs are
summed on the host (the "all-reduce").
"""

import sys

if "/opt/trn_rl_repo" not in sys.path:
    sys.path.insert(0, "/opt/trn_rl_repo")

import numpy as np
import ml_dtypes

BF = ml_dtypes.bfloat16

S = 2048
D = 2048
H = 16
HK = 4
HD = 128
WIN = 512
THETA = 10000.0
N_CORES = 8
NQT = S // 128          # 16 q row-tiles
NKT = D // 128          # 16 contraction tiles for the projections
WMAX = WIN + 128        # 640: max key-window width per q-tile
NEG = -1.0e30

_CACHE = {}


def _tables():
    """RoPE tables [128, S] fp16 with head-dim-duplicated frequencies
    (row p uses invf[p % 64]). The sin table has the rotate-half sign
    folded in and lives at the partition of the SOURCE operand: rows
    64:128 carry -sin (read together with ps[64:128] to produce the low
    output half), rows 0:64 carry +sin. Q tables are pre-scaled by the
    attention scale 1/sqrt(64)."""
    invf = 1.0 / (THETA ** (np.arange(0, HD, 2, dtype=np.float64) / HD))  # [64]
    t = np.arange(S, dtype=np.float64)
    fr = np.outer(invf, t)  # [64, S]
    cosf = np.concatenate([np.cos(fr)] * 2, axis=0)
    sinf = np.concatenate([np.sin(fr), -np.sin(fr)], axis=0)
    return (np.ascontiguousarray(cosf * 0.125, dtype=np.float16),
            np.ascontiguousarray(sinf * 0.125, dtype=np.float16),
            np.ascontiguousarray(cosf, dtype=np.float16),
            np.ascontiguousarray(sinf, dtype=np.float16))


def _masks():
    p = np.arange(128)[:, None]
    c = np.arange(WMAX)[None, :]
    band = (c - p >= 1) & (c - p <= WIN)
    mw = np.where(band, 0.0, NEG).astype(BF)          # [128, 640]
    cc = np.arange(128)[None, :]
    mc = np.where(cc <= p, 0.0, NEG).astype(BF)       # [128, 128] causal
    # edge mask: cols [0,512) allowed, cols [512,640) causal triangle.
    # slicing the last w cols gives the mask for edge q-tiles (qi < 4).
    me = np.zeros((128, WMAX), dtype=BF)
    me[:, WIN:] = mc
    return mw, me


def _build_program():
    import concourse.bacc as bacc
    import concourse.tile as tile
    from concourse import mybir

    bf = mybir.dt.bfloat16
    f32 = mybir.dt.float32
    EXP = mybir.ActivationFunctionType.Exp
    RELU = mybir.ActivationFunctionType.Relu
    MULT = mybir.AluOpType.mult
    ADD = mybir.AluOpType.add
    MAX = mybir.AluOpType.max
    DIV = mybir.AluOpType.divide

    nc = bacc.Bacc("TRN2", target_bir_lowering=False, debug=False,
                   num_devices=N_CORES)

    xt_d = nc.dram_tensor("xt", [128, NKT, S], bf, kind="ExternalInput")
    wq_d = nc.dram_tensor("wq", [128, NKT, 2, 128], bf, kind="ExternalInput")
    wk_d = nc.dram_tensor("wk", [128, NKT, 128], bf, kind="ExternalInput")
    wv_d = nc.dram_tensor("wv", [128, NKT, 128], bf, kind="ExternalInput")
    wo_d = nc.dram_tensor("wo", [128, 2, D], bf, kind="ExternalInput")
    lamn_d = nc.dram_tensor("lamn", [1, 2], f32, kind="ExternalInput")
    f16 = mybir.dt.float16
    out_d = nc.dram_tensor("outp", [S, D], f16, kind="ExternalOutput")

    tqc_np, tqs_np, tkc_np, tks_np = _tables()
    mw_np, me_np = _masks()
    tqc_d = nc.inline_tensor(tqc_np, "tab_qc")
    tqs_d = nc.inline_tensor(tqs_np, "tab_qs")
    tkc_d = nc.inline_tensor(tkc_np, "tab_kc")
    tks_d = nc.inline_tensor(tks_np, "tab_ks")
    mw_d = nc.inline_tensor(mw_np, "mask_win")
    me_d = nc.inline_tensor(me_np, "mask_edge")
    idb_d = nc.inline_tensor(np.eye(128, dtype=BF), "ident_bf")
    idf_d = nc.inline_tensor(np.eye(128, dtype=np.float32), "ident_f32")

    with tile.TileContext(nc) as tc:
        with tc.tile_pool(name="xpool", bufs=1) as xp, \
             tc.tile_pool(name="wpool", bufs=1) as wp, \
             tc.tile_pool(name="pers", bufs=1) as pers:

            # DMA issue order matters: projection weights + first x chunk
            # first so matmuls start early; bulk x + tables stream in under
            # compute; phase-2/3 constants (masks, wo) last.
            wq = wp.tile([128, NKT, 2, 128], bf)
            wk = wp.tile([128, NKT, 128], bf)
            wv = wp.tile([128, NKT, 128], bf)
            idb = wp.tile([128, 128], bf)
            lamn = wp.tile([1, 2], f32)

            xts = []
            for nch in range(4):
                xc = xp.tile([128, NKT, 512], bf, tag=f"xt{nch}")
                xts.append(xc)
            tqc = wp.tile([128, S], f16)
            tqs = wp.tile([128, S], f16)
            tkc = wp.tile([128, S], f16)
            tks = wp.tile([128, S], f16)

            def load_chunk(nch, ndma):
                sl = slice(nch * 512, (nch + 1) * 512)
                kstep = NKT // ndma
                for i in range(ndma):
                    ksl = slice(i * kstep, (i + 1) * kstep)
                    nc.sync.dma_start(out=xts[nch][:, ksl, :],
                                      in_=xt_d[:, ksl, sl])

            for i in range(4):
                nc.sync.dma_start(out=wq[:, 4 * i:4 * i + 4],
                                  in_=wq_d[:, 4 * i:4 * i + 4])
                if i == 0:
                    nc.sync.dma_start(out=wk[:, 0:4], in_=wk_d[:, 0:4])
                    nc.sync.dma_start(out=wv[:, 0:4], in_=wv_d[:, 0:4])
                for j in range(2):
                    k2 = slice(4 * i + 2 * j, 4 * i + 2 * (j + 1))
                    nc.sync.dma_start(out=xts[0][:, k2, :],
                                      in_=xt_d[:, k2, 0:512])
            nc.sync.dma_start(out=wk[:, 4:16], in_=wk_d[:, 4:16])
            nc.sync.dma_start(out=wv[:, 4:16], in_=wv_d[:, 4:16])
            nc.sync.dma_start(out=idb[:], in_=idb_d[:])
            nc.sync.dma_start(out=lamn[:], in_=lamn_d[:])
            for i in range(2):
                sl = slice(1024 * i, 1024 * (i + 1))
                nc.sync.dma_start(out=tqc[:, sl], in_=tqc_d[:, sl])
                nc.sync.dma_start(out=tqs[:, sl], in_=tqs_d[:, sl])
                nc.sync.dma_start(out=tkc[:, sl], in_=tkc_d[:, sl])
                nc.sync.dma_start(out=tks[:, sl], in_=tks_d[:, sl])
            load_chunk(1, 8)
            load_chunk(2, 4)
            load_chunk(3, 4)
            mw = wp.tile([128, WMAX], bf)
            nc.sync.dma_start(out=mw[:], in_=mw_d[:])
            me = wp.tile([128, WMAX], bf)
            nc.sync.dma_start(out=me[:], in_=me_d[:])
            wo = wp.tile([128, 2, D], bf)
            for i in range(4):
                nc.sync.dma_start(out=wo[:, :, 512 * i:512 * (i + 1)],
                                  in_=wo_d[:, :, 512 * i:512 * (i + 1)])
            lamb = wp.tile([128, 2], f32)
            nc.gpsimd.partition_broadcast(lamb[:], lamn[:])

            # q stored zero-padded to full 128 contraction rows per half:
            # qtp0 rows 0:64 hold half-0 q, rows 64:128 are zero; qtp1 is
            # the mirror. A 128-contraction matmul runs at 2x the column
            # rate of a 64-contraction one, so the padded zeros are free.
            qtp0 = pers.tile([128, 2, S], bf)
            qtp1 = pers.tile([128, 2, S], bf)
            kt = pers.tile([128, S], bf)         # RoPE'd k, hd-major
            vsm = pers.tile([128, NQT, 128], bf)  # v, S-major [s, hd]
            att = pers.tile([128, 2, S], bf)     # attention out^T, hd-major
            zeros = pers.tile([128, WMAX], bf)
            nc.gpsimd.memset(qtp0[64:128, :, :], 0.0)
            nc.gpsimd.memset(qtp1[0:64, :, :], 0.0)
            nc.gpsimd.memset(zeros[:], 0.0)

            # ---- Phase 1: projections + RoPE + v transpose ----
            with tc.tile_pool(name="pp", bufs=1, space="PSUM") as pp, \
                 tc.tile_pool(name="pt", bufs=2) as pt:
                for nch in range(4):
                    sl = slice(nch * 512, (nch + 1) * 512)
                    ps_q0 = pp.tile([128, 512], f32, tag="pq0", bufs=2)
                    ps_q1 = pp.tile([128, 512], f32, tag="pq1", bufs=2)
                    ps_k = pp.tile([128, 512], f32, tag="pk", bufs=1)
                    ps_v = pp.tile([128, 512], f32, tag="pv", bufs=1)
                    for kti in range(NKT):
                        st = kti == 0
                        sp = kti == NKT - 1
                        rhs = xts[nch][:, kti, :]
                        nc.tensor.matmul(ps_q0[:], wq[:, kti, 0, :], rhs, start=st, stop=sp)
                        nc.tensor.matmul(ps_q1[:], wq[:, kti, 1, :], rhs, start=st, stop=sp)
                        nc.tensor.matmul(ps_k[:], wk[:, kti, :], rhs, start=st, stop=sp)
                        nc.tensor.matmul(ps_v[:], wv[:, kti, :], rhs, start=st, stop=sp)
                    for ps, outlo, outhi, tabc, tabs, tg in (
                            (ps_q0, qtp0[0:64, 0, sl], qtp1[64:128, 0, sl], tqc, tqs, "q0"),
                            (ps_q1, qtp0[0:64, 1, sl], qtp1[64:128, 1, sl], tqc, tqs, "q1"),
                            (ps_k, None, None, tkc, tks, "k")):
                        # out = ps*cos + rot_half(ps)*sin, sign folded in
                        # tabs. The PSUM is cast to bf16 SBUF on the (idle
                        # in phase 1) scalar engine so the DVE muls/adds
                        # run in 2-byte 2x mode.
                        psb = pt.tile([128, 512], bf, tag="psb" + tg,
                                      name="psb")
                        nc.scalar.copy(out=psb[:], in_=ps[:])
                        m1 = pt.tile([128, 512], bf, tag="m1" + tg, name="m1")
                        m2 = pt.tile([128, 512], bf, tag="m2" + tg, name="m2")
                        nc.vector.tensor_mul(m1[:], psb[:], tabc[:, sl])
                        nc.vector.tensor_mul(m2[0:64, :], psb[64:128, :], tabs[64:128, sl])
                        nc.vector.tensor_mul(m2[64:128, :], psb[0:64, :], tabs[0:64, sl])
                        if outlo is None:
                            nc.vector.tensor_add(kt[:, sl], m1[:], m2[:])
                        else:
                            nc.vector.tensor_add(outlo, m1[0:64, :], m2[0:64, :])
                            nc.vector.tensor_add(outhi, m1[64:128, :], m2[64:128, :])
                    vtmp = pt.tile([128, 512], bf, tag="vtmp")
                    nc.vector.tensor_copy(out=vtmp[:], in_=ps_v[:])
                    nc.sync.dma_start(out=vsm[:, 4 * nch:4 * (nch + 1), :],
                                      in_=vtmp[:], transpose=True)

            # ---- Phase 2: attention ----
            with tc.tile_pool(name="psc", bufs=1, space="PSUM") as psc, \
                 tc.tile_pool(name="pse", bufs=1) as pse, \
                 tc.tile_pool(name="psm", bufs=1) as psm:
                for qi in range(NQT):
                    qsl = slice(qi * 128, (qi + 1) * 128)
                    kw = min(qi + 1, 5)
                    w = kw * 128
                    kstart = max(0, qi - 4)
                    kwin = slice(kstart * 128, kstart * 128 + w)

                    # both heads' AV share one 256-wide PSUM tile
                    ps_av = psc.tile([128, 2, 128], f32, tag="av", bufs=1)
                    gts = pse.tile([128, 2, 5, 128], bf, tag="gts", bufs=2)

                    for h in range(2):
                        ps_s1 = psc.tile([128, WMAX], f32, tag="s", bufs=2)
                        ps_s2 = psc.tile([128, WMAX], f32, tag="s", bufs=2)
                        for ps, lhsq in ((ps_s1, qtp0), (ps_s2, qtp1)):
                            lhs = lhsq[:, h, qsl]
                            if qi >= 4:
                                # causal edge lives only in block 0, so the
                                # mask matmul covers 128 cols; the score
                                # matmul splits at that boundary
                                nc.tensor.matmul(ps[:, 0:128], idb[:], mw[:, 0:128],
                                                 start=True, stop=False)
                                nc.tensor.matmul(ps[:, 0:128], lhs,
                                                 kt[:, kwin][:, 0:128],
                                                 start=False, stop=True)
                                nc.tensor.matmul(ps[:, 128:512], lhs,
                                                 kt[:, kwin][:, 128:512],
                                                 start=True, stop=True)
                                nc.tensor.matmul(ps[:, 512:640], idb[:],
                                                 mw[:, 512:640],
                                                 start=True, stop=False)
                                nc.tensor.matmul(ps[:, 512:640], lhs,
                                                 kt[:, kwin][:, 512:640],
                                                 start=False, stop=True)
                            else:
                                nc.tensor.matmul(ps[:, 0:w], idb[:],
                                                 me[:, WMAX - w:WMAX],
                                                 start=True, stop=False)
                                nc.tensor.matmul(ps[:, 0:w], lhs,
                                                 kt[:, kwin][:, 0:w],
                                                 start=False, stop=True)

                        e1 = pse.tile([128, WMAX], bf, tag="e1", bufs=2)
                        e2 = pse.tile([128, WMAX], bf, tag="e2", bufs=2)
                        s1 = psm.tile([128, 1], f32, tag="s1", bufs=4)
                        s2 = psm.tile([128, 1], f32, tag="s2", bufs=4)
                        nc.scalar.activation(out=e1[:, 0:w], in_=ps_s1[:, 0:w],
                                             func=EXP, accum_out=s1[:])
                        nc.scalar.activation(out=e2[:, 0:w], in_=ps_s2[:, 0:w],
                                             func=EXP, accum_out=s2[:])

                        # cneg = -(lam * s1 / s2)   (lamn holds -lam)
                        r2 = psm.tile([128, 1], f32, tag="r2", bufs=4)
                        nc.vector.reciprocal(out=r2[:], in_=s2[:])
                        cneg = psm.tile([128, 1], f32, tag="cneg", bufs=4)
                        nc.vector.scalar_tensor_tensor(
                            out=cneg[:], in0=s1[:], scalar=lamb[:, h:h + 1],
                            in1=r2[:], op0=MULT, op1=MULT)
                        # g0 = e1 + cneg*e2 ; g = relu(g0), accum D'
                        g0 = pse.tile([128, WMAX], bf, tag="g0", bufs=2)
                        g = pse.tile([128, WMAX], bf, tag="g", bufs=2)
                        dsum = psm.tile([128, 1], f32, tag="dsum", bufs=4)
                        nc.vector.scalar_tensor_tensor(
                            out=g0[:, 0:w], in0=e2[:, 0:w], scalar=cneg[:],
                            in1=e1[:, 0:w], op0=MULT, op1=ADD)
                        if h == 0:
                            nc.scalar.activation(
                                out=g[:, 0:w], in_=g0[:, 0:w], func=RELU,
                                accum_out=dsum[:])
                        else:
                            nc.vector.tensor_scalar(
                                out=g[:, 0:w], in0=g0[:, 0:w], scalar1=0.0,
                                scalar2=0.0, op0=MAX, op1=ADD, accum_out=dsum[:])
                        # recd = 1 / (D' + 1e-6 * s1); gn = g * recd
                        dtmp = psm.tile([128, 1], f32, tag="dtmp", bufs=4)
                        nc.vector.scalar_tensor_tensor(
                            out=dtmp[:], in0=s1[:], scalar=1e-6, in1=dsum[:],
                            op0=MULT, op1=ADD)
                        recd = psm.tile([128, 1], f32, tag="recd", bufs=4)
                        nc.vector.reciprocal(out=recd[:], in_=dtmp[:])
                        gn = pse.tile([128, WMAX], bf, tag="gn", bufs=2)
                        nc.vector.tensor_scalar(
                            out=gn[:, 0:w], in0=g[:, 0:w], scalar1=recd[:],
                            scalar2=0.0, op0=MULT, op1=ADD)

                        # transpose gn -> gT (PSUM) -> SBUF
                        ps_tr = psc.tile([128, kw, 128], bf, tag="trg", bufs=2)
                        for j in range(kw):
                            nc.tensor.transpose(ps_tr[:, j, :],
                                                gn[:, 128 * j:128 * (j + 1)], idb[:])
                        if h == 0:
                            nc.vector.tensor_copy(out=gts[:, 0, 0:kw, :], in_=ps_tr[:])
                        else:
                            nc.scalar.copy(out=gts[:, 1, 0:kw, :], in_=ps_tr[:])

                    # AV for both heads at once: [k,hd]^T-contract x [k, 2*128]
                    for j in range(kw):
                        nc.tensor.matmul(ps_av[:], vsm[:, kstart + j, :],
                                         gts[:, :, j, :],
                                         start=(j == 0), stop=(j == kw - 1))

                    nc.vector.tensor_copy(out=att[:, :, qsl], in_=ps_av[:])

                    # out-projection for this q-tile, interleaved so the PE
                    # fills attention bubbles and the output DMA spreads out
                    so = pse.tile([128, 2048], f16, tag="so", bufs=2)
                    for dch in range(4):
                        dsl = slice(dch * 512, (dch + 1) * 512)
                        ps_o = psc.tile([128, 512], f32, tag="o", bufs=1)
                        nc.tensor.matmul(ps_o[:], att[:, 0, qsl], wo[:, 0, dsl],
                                         start=True, stop=False)
                        nc.tensor.matmul(ps_o[:], att[:, 1, qsl], wo[:, 1, dsl],
                                         start=False, stop=True)
                        if dch % 2 == 0:
                            nc.vector.tensor_copy(out=so[:, dsl], in_=ps_o[:])
                        else:
                            nc.scalar.copy(out=so[:, dsl], in_=ps_o[:])
                        if dch % 2 == 1:
                            dsl2 = slice((dch - 1) * 512, (dch + 1) * 512)
                            nc.sync.dma_start(out=out_d[qsl, dsl2], in_=so[:, dsl2])

    nc.compile()
    return nc


def get_program():
    if "nc" not in _CACHE:
        _CACHE["nc"] = _build_program()
    return _CACHE["nc"]


def _prep_inputs(x, Wq, Wk, Wv, Wo, lam):
    xt = np.ascontiguousarray(x.reshape(S, D).T.astype(BF)
                              .reshape(NKT, 128, S).transpose(1, 0, 2))
    in_maps = []
    for c in range(N_CORES):
        h0 = 2 * c
        kv = c // 2
        wq_c = np.ascontiguousarray(
            Wq[:, h0 * 128:(h0 + 2) * 128].astype(BF)
            .reshape(NKT, 128, 2, 128).transpose(1, 0, 2, 3))
        wk_c = np.ascontiguousarray(
            Wk[:, kv * 128:(kv + 1) * 128].astype(BF)
            .reshape(NKT, 128, 128).transpose(1, 0, 2))
        wv_c = np.ascontiguousarray(
            Wv[:, kv * 128:(kv + 1) * 128].astype(BF)
            .reshape(NKT, 128, 128).transpose(1, 0, 2))
        wo_c = np.ascontiguousarray(
            Wo[h0 * 128:(h0 + 2) * 128, :].astype(BF)
            .reshape(2, 128, D).transpose(1, 0, 2))
        lamn_c = np.array([[-float(lam[h0]), -float(lam[h0 + 1])]], dtype=np.float32)
        in_maps.append({"xt": xt, "wq": wq_c, "wk": wk_c, "wv": wv_c,
                        "wo": wo_c, "lamn": lamn_c})
    return in_maps


def kernel(x, Wq, Wk, Wv, Wo, lam):
    from concourse.bass_utils import run_bass_kernel_spmd

    nc = get_program()
    in_maps = _prep_inputs(np.asarray(x), np.asarray(Wq), np.asarray(Wk),
                           np.asarray(Wv), np.asarray(Wo), np.asarray(lam))
    res = run_bass_kernel_spmd(nc, in_maps, list(range(N_CORES)))
    out = np.zeros((S, D), dtype=np.float32)
    for c in range(N_CORES):
        out += res.results[c]["outp"].astype(np.float32)
    return out.reshape(1, S, D)

